# revision 1
# baseline (speedup 1.0000x reference)
"""Trainium2 Bass kernel v4 for entmax-1.5 over rows of a masked [8192, 4096] matrix.

Candidate-set Newton (see kernel2/3/4), tuned per trace analysis:
  - group 0: closed-form top-8 warm start (one ACT sqrt while ACT is idle)
    + K=2 batched Newton; group 1: sqrt-free start tau0 = max(c1-1,
    (c1+c2)/2 - sqrt(1/2)) + K=3 pure-DVE Newton (keeps tau1 off the ACT
    queue and one iteration cheaper than the rowmax-1 start)
  - 32 candidates/row (4 chunks of 512 post-fold)
  - finals: relu on ACT (6 tiles) or DVE tensor_scalar (2 tiles); the square
    always on DVE (tensor_tensor fp16 2x) since ACT was the serial bottleneck
Numpy-validated rel err (incl. fp16 rounding): 7.58e-3 vs 2e-2 gate.

Sharding: 1024 rows x 8 cores; 8 tiles of [128, 4096] per core in 2 groups of 4.
Self-contained: hardcodes scores[8192,4096] f32 + mask[8192,4096] bool.
"""

import sys

import numpy as np

sys.path.insert(0, "/opt/trn_rl_repo")

N_ROWS = 8192
N_COLS = 4096
N_CORES = 8
P = 128
ROWS_PER_CORE = N_ROWS // N_CORES          # 1024
NT = ROWS_PER_CORE // P                    # 8 tiles per core
NCH = 4
CAND = NCH * 8                             # 32 candidates per row
K_NEWTON = 2
GROUPS = [(0, 1, 2, 3), (4, 5, 6, 7)]
CSPLIT = [3328, 3328, 3328, 3328, 1280, 1280, 1280, 1280]  # ACT does [0:c], DVE does [c:]

_CACHE = {}


def build_nc():
    import concourse.bacc as bacc
    import concourse.mybir as mybir
    from concourse.tile import TileContext
    from concourse.tile_rust import add_dep_helper

    def _raw(x):
        for attr in ("ins", "instruction", "inst"):
            if hasattr(x, attr):
                return getattr(x, attr)
        return x

    f32 = mybir.dt.float32
    f16 = mybir.dt.float16
    Alu = mybir.AluOpType
    Act = mybir.ActivationFunctionType

    nc = bacc.Bacc("TRN2", target_bir_lowering=False, debug=False)

    z_h = nc.declare_dram_parameter("z", [ROWS_PER_CORE, N_COLS], f16, isOutput=False)
    invk_h = nc.declare_dram_parameter("invk", [P, 8], f32, isOutput=False)
    kvec_h = nc.declare_dram_parameter("kvec", [P, 8], f32, isOutput=False)
    p_h = nc.declare_dram_parameter("p", [ROWS_PER_CORE, N_COLS], f16, isOutput=True)

    z = z_h.ap()
    pout = p_h.ap()
    half = N_COLS // 2
    csz = half // NCH                      # 512

    with TileContext(nc) as tc:
        with (
            tc.tile_pool(name="pt", bufs=NT) as pt,
            tc.tile_pool(name="pw", bufs=2) as pw,
            tc.tile_pool(name="pu", bufs=6) as pu,
            tc.tile_pool(name="pp", bufs=4) as pp,
            tc.tile_pool(name="ps", bufs=1) as ps,
            tc.tile_pool(name="pq", bufs=4) as pq,
        ):
            invk = ps.tile([P, 8], f32)
            kvec = ps.tile([P, 8], f32)
            tau = ps.tile([P, NT], f32, name="tau")
            nega = ps.tile([P, NT], f32, name="nega")

            t_tiles = [None] * NT
            c_tiles = {}
            t8_tiles = {}
            u_tiles = {}

            def phase_scan(gi, after=None):
                tiles = GROUPS[gi]
                g = len(tiles)
                C = ps.tile([P, g * CAND], f16, name=f"C{gi}")
                T8 = ps.tile([P, g * 8], f32, name=f"T8_{gi}")
                c_tiles[gi] = C
                t8_tiles[gi] = T8
                for j, i in enumerate(tiles):
                    t_i = pt.tile([P, N_COLS], f16, name=f"t{i}", tag="t")
                    nc.sync.dma_start(out=t_i, in_=z[i * P:(i + 1) * P, :])
                    t_tiles[i] = t_i
                    w = pw.tile([P, half], f16, name=f"w{i}", tag="w")
                    fold_inst = nc.vector.tensor_tensor(t_i[:, :half], t_i[:, :half] if False else t_i[:, :half], t_i[:, half:], Alu.max) if False else nc.vector.tensor_tensor(w, t_i[:, :half], t_i[:, half:], Alu.max)
                    if j == 0 and after is not None:
                        add_dep_helper(_raw(fold_inst), _raw(after), sync=False,
                                       reason="stage groups: solve g-1 before scan g")
                    for c in range(NCH):
                        nc.vector.max(
                            C[:, j * CAND + c * 8: j * CAND + (c + 1) * 8],
                            w[:, c * csz:(c + 1) * csz])
                    nc.vector.max(T8[:, j * 8:(j + 1) * 8], C[:, j * CAND:(j + 1) * CAND])

            def phase_solve(gi, k_newton=None, warm=True):
                tiles = GROUPS[gi]
                g = len(tiles)
                j0 = tiles[0]
                C = c_tiles[gi]
                T8 = t8_tiles[gi]
                tslice = tau[:, j0:j0 + g]
                sh3 = [P, g, 8]
                T3 = T8.rearrange("p (g k) -> p g k", g=g)
                invk_b = invk.rearrange("p (o k) -> p o k", o=1).broadcast_to(sh3)
                kvec_b = kvec.rearrange("p (o k) -> p o k", o=1).broadcast_to(sh3)
                if k_newton is None:
                    k_newton = K_NEWTON
                hp = tc.high_priority()
                hp.__enter__()
                if not warm:
                    # tau0 = max(c1 - 1, (c1+c2)/2 - sqrt(1/2)) — sqrt-free lower bound
                    T3w = T8.rearrange("p (g k) -> p g k", g=g)
                    tmp = pq.tile([P, g], f32, name=f"t0a_{gi}", tag=f"t0a_{gi}")
                    nc.vector.tensor_tensor(
                        tmp.rearrange("p (g o) -> p g o", o=1),
                        T3w[:, :, 0:1], T3w[:, :, 1:2], Alu.add)
                    nc.vector.tensor_scalar(tmp, tmp, 0.5, -0.70710678,
                                            Alu.mult, Alu.add)
                    nc.vector.tensor_scalar(
                        tslice, T3w[:, :, 0], -1.0, None, Alu.add)
                    nc.vector.tensor_tensor(tslice, tslice, tmp, Alu.max)
                # warm: closed-form entmax on sorted top-8
                if warm:
                    q8 = pq.tile(sh3, f32, name=f"q8_{gi}", tag=f"q8_{gi}")
                    nc.vector.tensor_tensor(q8, T3, T3, Alu.mult)

                    def cumsum8(src_, pref):
                        a1 = pq.tile(sh3, f32, name=f"{pref}a_{gi}", tag=f"{pref}a_{gi}")
                        nc.vector.tensor_copy(a1[:, :, 0:1], src_[:, :, 0:1])
                        nc.vector.tensor_tensor(a1[:, :, 1:8], src_[:, :, 1:8], src_[:, :, 0:7], Alu.add)
                        a2 = pq.tile(sh3, f32, name=f"{pref}b_{gi}", tag=f"{pref}b_{gi}")
                        nc.vector.tensor_copy(a2[:, :, 0:2], a1[:, :, 0:2])
                        nc.vector.tensor_tensor(a2[:, :, 2:8], a1[:, :, 2:8], a1[:, :, 0:6], Alu.add)
                        a4 = pq.tile(sh3, f32, name=f"{pref}c_{gi}", tag=f"{pref}c_{gi}")
                        nc.vector.tensor_copy(a4[:, :, 0:4], a2[:, :, 0:4])
                        nc.vector.tensor_tensor(a4[:, :, 4:8], a2[:, :, 4:8], a2[:, :, 0:4], Alu.add)
                        return a4

                    cs = cumsum8(T3, "cs")
                    cq = cumsum8(q8, "cq")
                    mean = pq.tile(sh3, f32, name=f"mean_{gi}", tag=f"mean_{gi}")
                    nc.vector.tensor_tensor(mean, cs, invk_b, Alu.mult)
                    mm = pq.tile(sh3, f32, name=f"mm_{gi}", tag=f"mm_{gi}")
                    nc.vector.tensor_tensor(mm, cq, invk_b, Alu.mult)
                    m2 = pq.tile(sh3, f32, name=f"m2_{gi}", tag=f"m2_{gi}")
                    nc.vector.tensor_tensor(m2, mean, mean, Alu.mult)
                    nc.vector.tensor_tensor(m2, mm, m2, Alu.subtract)
                    nc.vector.tensor_tensor(m2, m2, kvec_b, Alu.mult)
                    nc.vector.tensor_scalar(m2, m2, -1.0, 1.0, Alu.mult, Alu.add)
                    nc.vector.tensor_tensor(m2, m2, invk_b, Alu.mult)
                    nc.vector.tensor_scalar(m2, m2, 0.0, None, Alu.max)
                    sq = pq.tile(sh3, f32, name=f"sq_{gi}", tag=f"sq_{gi}")
                    nc.scalar.sqrt(sq, m2)
                    tauc = pq.tile(sh3, f32, name=f"tauc_{gi}", tag=f"tauc_{gi}")
                    nc.vector.tensor_tensor(tauc, mean, sq, Alu.subtract)
                    ind = pq.tile(sh3, f32, name=f"ind_{gi}", tag=f"ind_{gi}")
                    nc.vector.tensor_tensor(ind, tauc, T3, Alu.is_le)
                    sel = pq.tile(sh3, f32, name=f"sel_{gi}", tag=f"sel_{gi}")
                    nc.vector.tensor_copy(sel[:, :, 7:8], ind[:, :, 7:8])
                    nc.vector.tensor_tensor(sel[:, :, 0:7], ind[:, :, 0:7], ind[:, :, 1:8], Alu.subtract)
                    nc.vector.tensor_tensor(tauc, tauc, sel, Alu.mult)
                    nc.vector.reduce_sum(
                        tslice.rearrange("p (g o) -> p g o", o=1),
                        tauc, axis=mybir.AxisListType.X)
                for it in range(k_newton):
                    U = pq.tile([P, g * CAND], f16, name=f"U{gi}_{it}", tag="U")
                    for j, i in enumerate(tiles):
                        nc.vector.tensor_scalar(
                            U[:, j * CAND:(j + 1) * CAND],
                            C[:, j * CAND:(j + 1) * CAND], tau[:, i:i + 1], 0.0,
                            Alu.subtract, Alu.max)
                    SQ = pq.tile([P, g * CAND], f16, name=f"SQ{gi}_{it}", tag="SQ")
                    nc.vector.tensor_tensor(SQ, U, U, Alu.mult)
                    hF = pq.tile([P, 2 * g], f32, name=f"hF{gi}_{it}", tag="hF")
                    nc.vector.reduce_sum(
                        hF[:, 0:g].rearrange("p (g o) -> p g o", o=1),
                        U.rearrange("p (g c) -> p g c", g=g),
                        axis=mybir.AxisListType.X)
                    nc.vector.reduce_sum(
                        hF[:, g:2 * g].rearrange("p (g o) -> p g o", o=1),
                        SQ.rearrange("p (g c) -> p g c", g=g),
                        axis=mybir.AxisListType.X)
                    num = pq.tile([P, g], f32, name=f"num{gi}_{it}", tag="num")
                    nc.vector.tensor_scalar(num, hF[:, g:2 * g], -1.0, 0.5,
                                            Alu.add, Alu.mult)
                    rd = pq.tile([P, g], f32, name=f"rd{gi}_{it}", tag="rd")
                    nc.vector.reciprocal(rd, hF[:, 0:g])
                    nc.vector.tensor_tensor(num, num, rd, Alu.mult)
                    nc.vector.tensor_tensor(tslice, tslice, num, Alu.add)
                last = nc.vector.tensor_scalar(nega[:, j0:j0 + g], tslice,
                                               -1.0, None, Alu.mult)
                hp.__exit__(None, None, None)
                return last

            def final_act(gi):
                for i in GROUPS[gi]:
                    c = CSPLIT[i]
                    if c == 0:
                        continue
                    t_i = t_tiles[i]
                    u = u_tiles.get(i)
                    if u is None:
                        u = pu.tile([P, N_COLS], f16, name=f"u{i}", tag="u")
                        u_tiles[i] = u
                    nc.scalar.activation(u[:, :c], t_i[:, :c], Act.Relu,
                                         bias=nega[:, i:i + 1], scale=1.0)
                    nc.scalar.activation(u[:, :c], u[:, :c], Act.Square)
                    nc.sync.dma_start(out=pout[i * P:(i + 1) * P, :c], in_=u[:, :c])

            def final_dve(gi, after=None):
                for i in GROUPS[gi]:
                    c = CSPLIT[i]
                    if c >= N_COLS:
                        continue
                    t_i = t_tiles[i]
                    u = u_tiles.get(i)
                    if u is None:
                        u = pu.tile([P, N_COLS], f16, name=f"u{i}", tag="u")
                        u_tiles[i] = u
                    ts_inst = nc.vector.tensor_scalar(u[:, c:], t_i[:, c:], tau[:, i:i + 1],
                                                      0.0, Alu.subtract, Alu.max)
                    if after is not None:
                        add_dep_helper(_raw(ts_inst), _raw(after), sync=False,
                                       reason="dve finals after solve g1")
                        after = None
                    p_t = pp.tile([P, N_COLS - c], f16, name=f"pf{i}", tag=f"pf{N_COLS - c}")
                    nc.vector.tensor_tensor(p_t, u[:, c:], u[:, c:], Alu.mult)
                    nc.sync.dma_start(out=pout[i * P:(i + 1) * P, c:], in_=p_t)

            phase_scan(0)
            nc.sync.dma_start(out=invk, in_=invk_h.ap())
            nc.sync.dma_start(out=kvec, in_=kvec_h.ap())
            s0 = phase_solve(0, k_newton=2, warm=True)
            phase_scan(1, after=s0)
            final_act(0)
            s1 = phase_solve(1, k_newton=3, warm=False)
            final_act(1)
            final_dve(0, after=s1)
            final_dve(1)

    nc.compile()
    return nc


def _host_prep(scores, mask):
    s = np.asarray(scores, dtype=np.float32)
    zq = (np.float32(0.5) * s).astype(np.float16)
    z16 = np.where(np.asarray(mask), zq, np.float16(-4.0))
    k = np.arange(1, 9, dtype=np.float32)
    invk = np.tile(np.float32(1.0) / k, (P, 1)).astype(np.float32)
    kvec = np.tile(k, (P, 1)).astype(np.float32)
    return z16, invk, kvec


def run(scores: np.ndarray, mask: np.ndarray, trace: bool = False, **kw):
    from concourse.bass_utils import run_bass_kernel_spmd

    assert scores.shape == (N_ROWS, N_COLS) and mask.shape == (N_ROWS, N_COLS)
    if "nc" not in _CACHE:
        _CACHE["nc"] = build_nc()
    nc = _CACHE["nc"]

    z16, invk, kvec = _host_prep(scores, mask)
    rpc = ROWS_PER_CORE
    in_maps = [
        {"z": np.ascontiguousarray(z16[i * rpc:(i + 1) * rpc]),
         "invk": invk, "kvec": kvec}
        for i in range(N_CORES)
    ]
    res = run_bass_kernel_spmd(nc, in_maps, list(range(N_CORES)), trace=trace, **kw)
    out = np.concatenate([res.results[i]["p"] for i in range(N_CORES)], axis=0)
    return np.ascontiguousarray(out.astype(np.float32)), res


def kernel(scores: np.ndarray, mask: np.ndarray) -> np.ndarray:
    return run(scores, mask)[0]


if __name__ == "__main__":
    rng = np.random.default_rng(0)
    scores = rng.standard_normal((N_ROWS, N_COLS), dtype=np.float32)
    mask = rng.integers(0, 2, (N_ROWS, N_COLS)).astype(bool)
    out = kernel(scores, mask)
    print("out", out.shape, out.dtype, "rowsum", out.sum(-1)[:4])

